# revision 7
# baseline (speedup 1.0000x reference)
"""GNN message-passing kernel for Trainium2 (8 NeuronCores, edge-data-parallel).

Math (reference):
    eq = einsum('dnf,fr->ndr', Xs, Wq)                  # [N, D, R]
    ek = einsum('dnf,dfr->ndr', Xs, Wk)                 # [N, D, R]
    w_ij = sum_d eq[n_i]*ek[n_j]                        # [E, R]
    out  = mlp_w(w_ij) * mlp_t(t_ij)                    # [E, F]

Device strategy (per core, E/8 edges):
  - Host sorts edges by n_j; each core's shard covers a narrow n_j window, so
    the ek projection table is sharded (~3.5k nodes) while the eq table is
    fully replicated (all nodes). Both tables live in SBUF as bf16 rows of
    256 elements (d-major: [d0r0..d0r63, d1..., d3r63]), node n at partition
    n%128, free-dim stripe n//128.
  - Tables are built on-device: matmul  X_d^T[128f,128n].T @ W[128f,64r].
  - Per 2048-edge supertile: two SBUF-source dma_gathers (transpose mode)
    produce [128, 2, 2048] tiles where partition p, chunk c holds row element
    c*128+p, edges on the free axis. Elementwise eq*ek then feeds the first
    MLP-W matmul directly: the degree-reduction is absorbed by replicating
    mlp_w1 rows (lhsT = vstack([w1, w1])), accumulating both chunks in PSUM.
  - mlp_t runs from a pre-transposed t tile; final elementwise product is
    written back as bf16 [128f, edges] and un-permuted on the host.
"""

import sys

if "/opt/trn_rl_repo" not in sys.path:
    sys.path.insert(0, "/opt/trn_rl_repo")

import ml_dtypes
import numpy as np

BF16 = ml_dtypes.bfloat16

# Problem dims (hardcoded per spec nn_HTR_7464653160731)
D, N, F, R, E, H = 4, 20000, 128, 64, 320000, 128
NCORES = 8

# Tiling
NSUP_NODES = 512          # node-build supertile
SUP = 2048                # edge gather supertile
SUB = 512                 # matmul subtile (PSUM free dim)
EK_WIN = 3584             # per-core ek node window (7 build supertiles)

_prog_cache = {}


def _build_program(npad, ek_win, epad, b2t_nonzero, b2w_nonzero=False):
    import concourse.bacc as bacc
    import concourse.mybir as mybir
    import concourse.tile as tile
    from concourse.library_config import mlp as mlp_lib

    f32 = mybir.dt.float32
    bf16 = mybir.dt.bfloat16
    i16 = mybir.dt.int16
    AF = mybir.ActivationFunctionType

    n_sup = npad // NSUP_NODES        # eq build supertiles
    ek_sup = ek_win // NSUP_NODES     # ek build supertiles
    n_esup = epad // SUP              # edge supertiles
    n_sub = SUP // SUB
    eq_ranks = npad // 128
    ek_ranks = ek_win // 128
    idx_cols = epad // 16

    nc = bacc.Bacc("TRN2", target_bir_lowering=False)

    # DRAM I/O
    xb = nc.dram_tensor("xb", [128, n_sup, D * NSUP_NODES], bf16, kind="ExternalInput")
    xw = nc.dram_tensor("xw", [128, ek_sup, D * NSUP_NODES], bf16, kind="ExternalInput")
    tt = nc.dram_tensor("tt", [128, epad], bf16, kind="ExternalInput")
    eqi = nc.dram_tensor("eqi", [128, idx_cols], i16, kind="ExternalInput")
    eki = nc.dram_tensor("eki", [128, idx_cols], i16, kind="ExternalInput")
    wq = nc.dram_tensor("wq", [128, R], bf16, kind="ExternalInput")
    wk = nc.dram_tensor("wk", [128, D * R], bf16, kind="ExternalInput")
    w1s = nc.dram_tensor("w1s", [128, H], bf16, kind="ExternalInput")
    mw2 = nc.dram_tensor("mw2", [128, F], bf16, kind="ExternalInput")
    mt1 = nc.dram_tensor("mt1", [128, H], bf16, kind="ExternalInput")
    mt2 = nc.dram_tensor("mt2", [128, F], bf16, kind="ExternalInput")
    bias = nc.dram_tensor("bias", [128, 4], f32, kind="ExternalInput")
    out = nc.dram_tensor("out", [128, epad], bf16, kind="ExternalOutput")

    with tile.TileContext(nc) as tc:
        nc.gpsimd.load_library(mlp_lib)
        with (
            tc.tile_pool(name="const", bufs=1) as cpool,
            tc.tile_pool(name="table", bufs=1) as tabp,
            tc.tile_pool(name="idx", bufs=1) as idxp,
        ):
            wq_sb = cpool.tile([128, R], bf16)
            wk_sb = cpool.tile([128, D * R], bf16)
            w1s_sb = cpool.tile([128, H], bf16)
            mw2_sb = cpool.tile([128, F], bf16)
            mt1_sb = cpool.tile([128, H], bf16)
            mt2_sb = cpool.tile([128, F], bf16)
            bias_sb = cpool.tile([128, 4], f32)
            for sb_t, dr in (
                (wq_sb, wq), (wk_sb, wk), (w1s_sb, w1s), (mw2_sb, mw2),
                (mt1_sb, mt1), (mt2_sb, mt2), (bias_sb, bias),
            ):
                nc.sync.dma_start(sb_t[:], dr[:])

            eqtab = tabp.tile([128, eq_ranks * 256], bf16)
            ektab = tabp.tile([128, ek_ranks * 256], bf16)
            eqi_sb = idxp.tile([128, idx_cols], i16)
            eki_sb = idxp.tile([128, idx_cols], i16)
            nc.sync.dma_start(eqi_sb[:], eqi[:])
            nc.sync.dma_start(eki_sb[:], eki[:])

            # ---- Phase A: build projection tables ----
            with (
                tc.tile_pool(name="xbuf", bufs=3) as xbuf,
                tc.tile_pool(name="bldp", bufs=4, space="PSUM") as bldp,
            ):
                def build(src, n_super, tab, weight_ap):
                    for s2 in range(n_super):
                        xt = xbuf.tile([128, D * NSUP_NODES], bf16, tag="xt")
                        nc.sync.dma_start(xt[:], src[:, s2, :])
                        for nt in range(NSUP_NODES // 128):
                            bld = bldp.tile([128, D, R], f32, tag="bld")
                            for d in range(D):
                                nc.tensor.matmul(
                                    bld[:, d, :],
                                    xt[:, d * NSUP_NODES + nt * 128:
                                       d * NSUP_NODES + (nt + 1) * 128],
                                    weight_ap(d),
                                    start=True, stop=True,
                                )
                            stripe = s2 * (NSUP_NODES // 128) + nt
                            nc.any.tensor_copy(
                                tab[:, stripe * 256:(stripe + 1) * 256]
                                .rearrange("p (d r) -> p d r", d=D),
                                bld[:, :, :],
                            )

                build(xb, n_sup, eqtab, lambda d: wq_sb[:])
                build(xw, ek_sup, ektab, lambda d: wk_sb[:, d * R:(d + 1) * R])

            # ---- Phase B: edge pipeline ----
            with (
                tc.tile_pool(name="gbuf", bufs=2) as gbuf,
                tc.tile_pool(name="tbuf", bufs=2) as tbuf,
                tc.tile_pool(name="hbuf", bufs=3) as hbuf,
                tc.tile_pool(name="swb", bufs=3) as swb,
                tc.tile_pool(name="obuf", bufs=2) as obuf,
                tc.tile_pool(name="psum", bufs=2, space="PSUM") as psum,
            ):
                for k in range(n_esup):
                    eqg = gbuf.tile([128, 2, SUP], bf16, tag="eqg")
                    ekg = gbuf.tile([128, 2, SUP], bf16, tag="ekg")
                    tt_t = tbuf.tile([128, SUP], bf16, tag="tt")
                    nc.sync.dma_start(tt_t[:], tt[:, k * SUP:(k + 1) * SUP])
                    nc.gpsimd.dma_gather(
                        eqg[:], eqtab[:], eqi_sb[:, k * (SUP // 16):(k + 1) * (SUP // 16)],
                        SUP, SUP, 256, transpose=True, single_packet=False,
                        sbuf_tokens_per_rank=128, sbuf_free_dim_per_rank=512,
                    )
                    nc.gpsimd.dma_gather(
                        ekg[:], ektab[:], eki_sb[:, k * (SUP // 16):(k + 1) * (SUP // 16)],
                        SUP, SUP, 256, transpose=True, single_packet=False,
                        sbuf_tokens_per_rank=128, sbuf_free_dim_per_rank=512,
                    )
                    nc.vector.tensor_mul(eqg[:], eqg[:], ekg[:])
                    ot = obuf.tile([128, SUP], bf16, tag="ot")
                    for j in range(n_sub):
                        sl = slice(j * SUB, (j + 1) * SUB)
                        p1w = psum.tile([128, SUB], f32, tag="p1w")
                        nc.tensor.matmul(p1w[:], w1s_sb[:], eqg[:, 0, sl],
                                         start=True, stop=False)
                        nc.tensor.matmul(p1w[:], w1s_sb[:], eqg[:, 1, sl],
                                         start=False, stop=True)
                        hw_t = hbuf.tile([128, SUB], bf16, tag="hw")
                        nc.scalar.activation(hw_t[:], p1w[:], AF.Relu,
                                             bias=bias_sb[:, 0:1])
                        p2w = psum.tile([128, SUB], f32, tag="p2w")
                        nc.tensor.matmul(p2w[:], mw2_sb[:], hw_t[:],
                                         start=True, stop=True)
                        p1t = psum.tile([128, SUB], f32, tag="p1t")
                        nc.tensor.matmul(p1t[:], mt1_sb[:], tt_t[:, sl],
                                         start=True, stop=True)
                        ht_t = hbuf.tile([128, SUB], bf16, tag="ht")
                        nc.scalar.activation(ht_t[:], p1t[:], AF.Relu,
                                             bias=bias_sb[:, 1:2])
                        p2t = psum.tile([128, SUB], f32, tag="p2t")
                        nc.tensor.matmul(p2t[:], mt2_sb[:], ht_t[:],
                                         start=True, stop=True)
                        sw_t = swb.tile([128, SUB], f32, tag="sw")
                        if b2w_nonzero:
                            nc.scalar.activation(sw_t[:], p2w[:], AF.Identity,
                                                 bias=bias_sb[:, 2:3])
                        else:
                            nc.scalar.activation(sw_t[:], p2w[:], AF.Copy)
                        if b2t_nonzero:
                            st_t = swb.tile([128, SUB], f32, tag="st")
                            nc.scalar.activation(st_t[:], p2t[:], AF.Identity,
                                                 bias=bias_sb[:, 3:4])
                            nc.vector.tensor_mul(ot[:, sl], st_t[:], sw_t[:])
                        else:
                            nc.vector.tensor_mul(ot[:, sl], p2t[:], sw_t[:])
                    nc.sync.dma_start(out[:, k * SUP:(k + 1) * SUP], ot[:])

    nc.compile()
    return nc


def get_program(npad, ek_win, epad, b2t_nonzero, b2w_nonzero):
    key = (npad, ek_win, epad, b2t_nonzero, b2w_nonzero)
    if key not in _prog_cache:
        _prog_cache[key] = _build_program(npad, ek_win, epad, b2t_nonzero,
                                          b2w_nonzero)
    return _prog_cache[key]


def _wrap_idxs(idx_pad, sup=SUP):
    """[epad] -> [128, epad//16] int16 in the per-gather 16-partition wrap."""
    n_sup = idx_pad.shape[0] // sup
    w = idx_pad.reshape(n_sup, sup // 16, 16).transpose(2, 0, 1).reshape(16, -1)
    return np.ascontiguousarray(np.tile(w, (8, 1)).astype(np.int16))


def _build_x_layout(xp_f32, nsup):
    """[D, nodes, F] f32 -> [128, nsup, D*512] bf16 with free=(d, j)."""
    f = xp_f32.shape[2]
    xt = xp_f32.transpose(2, 1, 0)                      # [F, nodes, D]
    xt = xt.reshape(f, nsup, NSUP_NODES, D).transpose(0, 1, 3, 2)
    return np.ascontiguousarray(xt.reshape(f, nsup, D * NSUP_NODES).astype(BF16))


def kernel(Xs, t_ij, edge_index, Wq, Wk, mw_w1, mw_b1, mw_w2, mw_b2,
           mt_w1, mt_b1, mt_w2, mt_b2):
    from concourse.bass_utils import run_bass_kernel_spmd

    Xs = np.asarray(Xs, np.float32)
    t_ij = np.asarray(t_ij, np.float32)
    edge_index = np.asarray(edge_index)
    in_dtype = edge_index.dtype

    esh = E // NCORES                      # edges per core
    epad = ((esh + SUP - 1) // SUP) * SUP
    npad = ((N + NSUP_NODES - 1) // NSUP_NODES) * NSUP_NODES

    # Sort edges by n_j so each core's ek window is narrow.
    # NOTE reference order: n_j, n_i = edge_index[0], edge_index[1]
    nj = edge_index[0].astype(np.int64)
    ni = edge_index[1].astype(np.int64)
    perm = np.argsort(nj, kind="stable")
    ni_s, nj_s, t_s = ni[perm], nj[perm], t_ij[perm]

    xp = np.zeros((D, npad, F), np.float32)
    xp[:, :N] = Xs
    xb_arr = _build_x_layout(xp, npad // NSUP_NODES)

    b2t_nonzero = bool(np.any(np.asarray(mt_b2) != 0))
    b2w_nonzero = bool(np.any(np.asarray(mw_b2) != 0))
    nc = get_program(npad, EK_WIN, epad, b2t_nonzero, b2w_nonzero)

    bias_arr = np.zeros((128, 4), np.float32)
    bias_arr[:, 0] = np.asarray(mw_b1, np.float32)
    bias_arr[:, 1] = np.asarray(mt_b1, np.float32)
    bias_arr[:, 2] = np.asarray(mw_b2, np.float32)
    bias_arr[:, 3] = np.asarray(mt_b2, np.float32)

    com = {
        "xb": xb_arr,
        "wq": np.ascontiguousarray(np.asarray(Wq).astype(BF16)),
        "wk": np.ascontiguousarray(
            np.asarray(Wk).transpose(1, 0, 2).reshape(F, D * R).astype(BF16)),
        "w1s": np.ascontiguousarray(
            np.vstack([np.asarray(mw_w1)] * 2).astype(BF16)),
        "mw2": np.ascontiguousarray(np.asarray(mw_w2).astype(BF16)),
        "mt1": np.ascontiguousarray(np.asarray(mt_w1).astype(BF16)),
        "mt2": np.ascontiguousarray(np.asarray(mt_w2).astype(BF16)),
        "bias": bias_arr,
    }

    in_maps = []
    for g in range(NCORES):
        s0, s1 = g * esh, (g + 1) * esh
        lo = min(int(nj_s[s0]) // 128 * 128, npad - EK_WIN)
        assert int(nj_s[s1 - 1]) - lo < EK_WIN, "ek window overflow"
        xw_arr = _build_x_layout(xp[:, lo:lo + EK_WIN], EK_WIN // NSUP_NODES)

        eq_idx = np.zeros(epad, np.int64)
        eq_idx[:esh] = ni_s[s0:s1]
        ek_idx = np.zeros(epad, np.int64)
        ek_idx[:esh] = nj_s[s0:s1] - lo

        tpad = np.zeros((epad, F), np.float32)
        tpad[:esh] = t_s[s0:s1]

        in_maps.append({
            **com,
            "xw": xw_arr,
            "tt": np.ascontiguousarray(tpad.T.astype(BF16)),
            "eqi": _wrap_idxs(eq_idx),
            "eki": _wrap_idxs(ek_idx),
        })

    res = run_bass_kernel_spmd(nc, in_maps, list(range(NCORES))).results

    sorted_out = np.empty((E, F), np.float32)
    for g in range(NCORES):
        o = np.asarray(res[g]["out"]).astype(np.float32)  # [128, epad]
        sorted_out[g * esh:(g + 1) * esh] = o[:, :esh].T

    result = np.empty((E, F), np.float32)
    result[perm] = sorted_out
    return result
